# revision 1
# baseline (speedup 1.0000x reference)
"""Trainium2 Bass kernel for nn_DZSpecimenClfToy.

Reference computation (per batch item b, B=8, one NeuronCore each):
  1. tv = bilinear_resize(topview[b], (3,64,64) -> (3,4,4))   # fixed 2x2 avg of rows/cols {7,8},{23,24},{39,40},{55,56}
  2. coords = sigmoid(tv.flat @ W1.T + b1).reshape(N,2)       # N=4096
  3. patch top-left tl = coords*2043 (+2-2); all 16 output px of a 4x4
     patch share one bilinear fraction pair -> 5x5 pixel support
  4. out[b] = bilinear_crops.flat @ W2.T + b2                 # [2]

Sharding: data-parallel over batch across 8 cores; weights replicated.

Gather strategy: the toolchain's indirect DMA only supports ONE address per
partition per instruction, so the host uploads the search view in an
overlapped-band layout: 511 bands of 8 rows (stride 4), each stored
[col][row_in_band][ch]. A patch's 5x5x3 support is then one contiguous
111-float run starting at triple index b*16384 + c0*8 + s (b=r0//4,
s=r0%4), and the extraction offsets d*24+(i+di)*3+c are independent of s.
4096 patches = 32 indirect DMAs of [128 partitions x 1 address].
"""
import functools
from contextlib import ExitStack

import numpy as np

import concourse.bass as bass
import concourse.tile as tile
from concourse import bacc, mybir
import concourse.bass_utils as bass_utils
from concourse.bass import IndirectOffsetOnAxis

F32 = mybir.dt.float32
I32 = mybir.dt.int32
ALU = mybir.AluOpType
ACT = mybir.ActivationFunctionType
AX = mybir.AxisListType

B = 8          # batch == number of cores
H = W = 2048   # search view height/width
N = 4096       # patches per item
PS = 4         # patch size
NCLS = 2       # classes
P = 128        # partitions
TPP = N // P   # patches per partition = 32

NBAND = 511            # bands of 8 rows, stride 4: rows 4b..4b+7
BANDTRIP = W * 8       # pixel-triples per band = 16384
SEG = 111              # gathered f32 per patch (covers d*24+(i+di)*3+c <= 110)
SEGP = 128             # SBUF stride per patch segment
MAGIC = 8388608.0      # 2**23


def build_program(num_devices: int, svh: int, svw: int, debug: bool = False):
    pad = float(svh - 1 - PS)  # 2043
    assert svh == H and svw == W, (svh, svw)

    nc = bacc.Bacc("TRN2", target_bir_lowering=False, debug=False,
                   enable_asserts=False, num_devices=num_devices)

    tv = nc.dram_tensor("tv", [3, 64, 64], F32, kind="ExternalInput").ap()
    svb = nc.dram_tensor("svb", [NBAND * BANDTRIP, 3], F32, kind="ExternalInput").ap()
    w1 = nc.dram_tensor("W1", [2 * N, 48], F32, kind="ExternalInput").ap()
    b1 = nc.dram_tensor("b1", [2 * N], F32, kind="ExternalInput").ap()
    w2 = nc.dram_tensor("W2p", [NCLS, N * PS * PS * 3], F32, kind="ExternalInput").ap()
    b2 = nc.dram_tensor("b2", [NCLS], F32, kind="ExternalInput").ap()
    out = nc.dram_tensor("out", [1, NCLS], F32, kind="ExternalOutput").ap()

    dbg = {}
    if debug:
        dbg["s"] = nc.dram_tensor("dbg_s", [P, 2 * TPP], F32, kind="ExternalOutput").ap()
        dbg["idx"] = nc.dram_tensor("dbg_idx", [P, TPP], I32, kind="ExternalOutput").ap()
        dbg["S"] = nc.dram_tensor("dbg_S", [P, TPP * SEGP], F32, kind="ExternalOutput").ap()
        dbg["U"] = nc.dram_tensor("dbg_U", [P, TPP * 48], F32, kind="ExternalOutput").ap()

    with tile.TileContext(nc) as tc:
        with ExitStack() as ctx:
            pool = ctx.enter_context(tc.tile_pool(name="main", bufs=1))

            # ---- input DMAs -------------------------------------------------
            # Topview rows {7,8},{23,24},{39,40},{55,56}: each pair is 128
            # contiguous floats starting at row 7 of each 16-row group.
            A = pool.tile([1, 1536], F32)          # [(c,k), r01*64]
            tv_sel = tv.rearrange("c (k s) w -> c k (s w)", s=16)[:, :, 7 * 64:9 * 64]
            nc.sync.dma_start(A[:].rearrange("p (c k x) -> p c k x", c=3, k=4),
                              tv_sel.unsqueeze(0))

            W1sb = pool.tile([P, 64 * 48], F32)    # row g=p*64+j at [p, j*48:...]
            nc.sync.dma_start(W1sb[:], w1.rearrange("(p j) k -> p (j k)", p=P))

            b1sb = pool.tile([P, 64], F32)
            nc.sync.dma_start(b1sb[:], b1.rearrange("(p j) -> p j", p=P))

            W2sb = pool.tile([P, NCLS * 1536], F32)  # [p, c*1536+x] = W2p[c, p*1536+x]
            nc.sync.dma_start(W2sb[:].rearrange("p (c x) -> p c x", c=NCLS),
                              w2.rearrange("c (p x) -> p c x", p=P))

            b2sb = pool.tile([1, NCLS], F32)
            nc.sync.dma_start(b2sb[:], b2.unsqueeze(0))

            # ---- topview 64x64 -> 4x4 resize, flatten, scale ---------------
            V = pool.tile([1, 768], F32)           # [(c,k), 64] row-pair sums
            A4 = A[:].rearrange("p (ck r w) -> p ck r w", ck=12, r=2)
            nc.vector.tensor_add(V[:].rearrange("p (ck w) -> p ck w", ck=12),
                                 A4[:, :, 0, :], A4[:, :, 1, :])
            F48 = pool.tile([1, 48], F32)
            V4 = V[:].rearrange("p (ck g s) -> p ck g s", ck=12, g=4)
            nc.vector.tensor_add(F48[:].rearrange("p (ck g) -> p ck g", ck=12),
                                 V4[:, :, :, 7], V4[:, :, :, 8])
            flatF = pool.tile([1, 48], F32)
            nc.vector.tensor_scalar_mul(flatF[:], F48[:], 0.25)

            # broadcast flat to all partitions (bounce through DRAM)
            dram_pool = ctx.enter_context(tc.tile_pool(name="dram", bufs=1, space="DRAM"))
            fdram = dram_pool.tile([1, 48], F32)
            nc.sync.dma_start(fdram[:], flatF[:])
            flatb = pool.tile([P, 48], F32)
            nc.sync.dma_start(flatb[:], fdram[:].to_broadcast((P, 48)))

            # ---- coords = sigmoid(W1 @ flat + b1), [128, 64] ---------------
            mul1 = pool.tile([P, 64 * 48], F32)
            nc.vector.tensor_mul(mul1[:].rearrange("p (j k) -> p j k", j=64),
                                 W1sb[:].rearrange("p (j k) -> p j k", j=64),
                                 flatb[:].unsqueeze(1).to_broadcast((P, 64, 48)))
            pre = pool.tile([P, 64], F32)
            nc.vector.reduce_sum(pre[:].unsqueeze(2),
                                 mul1[:].rearrange("p (j k) -> p j k", j=64),
                                 axis=AX.X)
            preb = pool.tile([P, 64], F32)
            nc.vector.tensor_add(preb[:], pre[:], b1sb[:])
            sg = pool.tile([P, 64], F32)
            nc.scalar.activation(sg[:], preb[:], ACT.Sigmoid)
            if debug:
                nc.sync.dma_start(dbg["s"], sg[:])

            # ---- patch top-left corners and fractions ----------------------
            s3 = sg[:].rearrange("p (t two) -> p t two", two=2)

            def floor_to(dst, src, tag):
                """dst = floor(src), src >= 0, via round-to-nearest + correction."""
                rnd = pool.tile([P, TPP], F32, tag=f"rnd{tag}")
                nc.vector.tensor_scalar(rnd[:], src, MAGIC, MAGIC,
                                        op0=ALU.add, op1=ALU.subtract)
                gt = pool.tile([P, TPP], F32, tag=f"gt{tag}")
                nc.vector.tensor_tensor(gt[:], rnd[:], src, op=ALU.is_gt)
                nc.vector.tensor_sub(dst, rnd[:], gt[:])

            def corner(sel):
                xs = pool.tile([P, TPP], F32, tag=f"xs{sel}")
                nc.vector.tensor_scalar(xs[:], s3[:, :, sel], pad, float(PS // 2),
                                        op0=ALU.mult, op1=ALU.add)
                tl = pool.tile([P, TPP], F32, tag=f"tl{sel}")
                nc.vector.tensor_scalar_sub(tl[:], xs[:], float(PS // 2))
                c0 = pool.tile([P, TPP], F32, tag=f"c0{sel}")
                floor_to(c0[:], tl[:], f"c{sel}")
                fr = pool.tile([P, TPP], F32, tag=f"fr{sel}")
                nc.vector.tensor_sub(fr[:], tl[:], c0[:])
                return c0, fr

            r0f, fr = corner(0)   # rows
            c0f, fc = corner(1)   # cols

            # ---- gather index (pixel-triple units, +MAGIC bias) ------------
            # band b = r0//4, s = r0%4, idx = b*16384 + c0*8 + s
            bq = pool.tile([P, TPP], F32)
            nc.vector.tensor_scalar_mul(bq[:], r0f[:], 0.25)
            bf = pool.tile([P, TPP], F32)
            floor_to(bf[:], bq[:], "b")
            sres = pool.tile([P, TPP], F32)        # s = r0 - 4b
            nc.vector.tensor_scalar(sres[:], bf[:], -4.0, None, op0=ALU.mult)
            nc.vector.tensor_add(sres[:], sres[:], r0f[:])
            t1 = pool.tile([P, TPP], F32)
            nc.vector.tensor_scalar(t1[:], bf[:], float(BANDTRIP), MAGIC,
                                    op0=ALU.mult, op1=ALU.add)
            t2 = pool.tile([P, TPP], F32)
            nc.vector.tensor_scalar(t2[:], c0f[:], 8.0, None, op0=ALU.mult)
            nc.vector.tensor_add(t2[:], t2[:], sres[:])
            idxf = pool.tile([P, TPP], F32)
            nc.vector.tensor_add(idxf[:], t1[:], t2[:])
            idxi = pool.tile([P, TPP], I32)
            nc.vector.tensor_single_scalar(idxi[:], idxf[:].bitcast(I32),
                                           0x007FFFFF, op=ALU.bitwise_and)
            if debug:
                nc.sync.dma_start(dbg["idx"], idxi[:])

            # ---- gather: one 111-float run per patch, 32 x [128 x 1] -------
            S = pool.tile([P, TPP * SEGP], F32)
            if debug:
                nc.vector.memset(S[:], 0.0)  # the dbg_S dump reads the padding
            for t in range(TPP):
                nc.gpsimd.indirect_dma_start(
                    out=S[:, t * SEGP: t * SEGP + SEG],
                    out_offset=None,
                    in_=svb,
                    in_offset=IndirectOffsetOnAxis(ap=idxi[:, t:t + 1], axis=0),
                )
            if debug:
                nc.sync.dma_start(dbg["S"], S[:])

            # ---- bilinear combine ------------------------------------------
            # segment layout per patch: elem(d, m, c) at d*24 + m*3 + c,
            # m = i + di (0..4). Row interp over di, col interp over dj.
            Sv = S[:].rearrange("p (t x) -> p t x", t=TPP)

            def seg_view(off):
                # [p, t, d(5 cols, stride 24), 12 = (i,c)] at element offset off
                return Sv[:, :, off:off + 120].rearrange(
                    "p t (d e) -> p t d e", d=5)[:, :, :, 0:12]

            D1 = pool.tile([P, TPP * 60], F32)
            D1v = D1[:].rearrange("p (t d e) -> p t d e", t=TPP, d=5)
            nc.vector.tensor_sub(D1v, seg_view(3), seg_view(0))
            M1 = pool.tile([P, TPP * 60], F32)
            M1v = M1[:].rearrange("p (t d e) -> p t d e", t=TPP, d=5)
            nc.vector.tensor_mul(M1v, D1v,
                                 fr[:].unsqueeze(2).unsqueeze(3).to_broadcast((P, TPP, 5, 12)))
            T = pool.tile([P, TPP * 60], F32)
            nc.vector.tensor_add(T[:].rearrange("p (t d e) -> p t d e", t=TPP, d=5),
                                 M1v, seg_view(0))

            # col interp: U[t, j, i, c] = T(d=j) + fc*(T(d=j+1) - T(d=j))
            Tv = T[:].rearrange("p (t x) -> p t x", t=TPP)
            T0 = Tv[:, :, 0:48].rearrange("p t (d e) -> p t d e", d=4)
            T12 = Tv[:, :, 12:60].rearrange("p t (d e) -> p t d e", d=4)
            D2 = pool.tile([P, TPP * 48], F32)
            D2v = D2[:].rearrange("p (t d e) -> p t d e", t=TPP, d=4)
            nc.vector.tensor_sub(D2v, T12, T0)
            M2 = pool.tile([P, TPP * 48], F32)
            M2v = M2[:].rearrange("p (t d e) -> p t d e", t=TPP, d=4)
            nc.vector.tensor_mul(M2v, D2v,
                                 fc[:].unsqueeze(2).unsqueeze(3).to_broadcast((P, TPP, 4, 12)))
            U = pool.tile([P, TPP * 48], F32)
            nc.vector.tensor_add(U[:].rearrange("p (t d e) -> p t d e", t=TPP, d=4),
                                 M2v, T0)
            if debug:
                nc.sync.dma_start(dbg["U"], U[:])

            # ---- classifier: out[c] = sum(U * W2p[c]) + b2 -----------------
            mW2 = pool.tile([P, NCLS * 1536], F32)
            nc.vector.tensor_mul(mW2[:].rearrange("p (c x) -> p c x", c=NCLS),
                                 W2sb[:].rearrange("p (c x) -> p c x", c=NCLS),
                                 U[:].unsqueeze(1).to_broadcast((P, NCLS, 1536)))
            r2 = pool.tile([P, NCLS], F32)
            nc.vector.reduce_sum(r2[:].unsqueeze(2),
                                 mW2[:].rearrange("p (c x) -> p c x", c=NCLS),
                                 axis=AX.X)
            ppool = ctx.enter_context(tc.tile_pool(name="ps", bufs=1, space="PSUM"))
            ones = pool.tile([P, 1], F32)
            nc.vector.memset(ones[:], 1.0)
            osum = ppool.tile([1, NCLS], F32)
            nc.tensor.matmul(out=osum[:], lhsT=ones[:], rhs=r2[:], start=True, stop=True)
            ofin = pool.tile([1, NCLS], F32)
            nc.vector.tensor_add(ofin[:], osum[:], b2sb[:])
            nc.sync.dma_start(out, ofin[:])

    nc.compile()
    return nc


@functools.lru_cache(maxsize=2)
def _compiled(num_devices: int, svh: int, svw: int, debug: bool = False):
    return build_program(num_devices, svh, svw, debug)


def band_layout(img: np.ndarray) -> np.ndarray:
    """[2048, 2048, 3] -> [511*16384, 3]: 8-row bands at stride 4, [col][row][ch]."""
    sw = np.lib.stride_tricks.sliding_window_view(img, 8, axis=0)  # [2041, 2048, 3, 8]
    sb = sw[::4]                                                   # [511, 2048, 3, 8]
    return np.ascontiguousarray(sb.transpose(0, 1, 3, 2)).reshape(-1, 3)


def permute_w2(W2: np.ndarray) -> np.ndarray:
    """Reorder per-patch (i, j, c) -> (j, i, c) to match the kernel's U layout."""
    return np.ascontiguousarray(
        W2.reshape(NCLS, N, PS, PS, 3).transpose(0, 1, 3, 2, 4)).reshape(NCLS, -1)


def make_in_maps(topview, search_views, W1, b1, W2, b2):
    W1 = np.ascontiguousarray(W1, np.float32)
    b1 = np.ascontiguousarray(b1, np.float32)
    W2p = permute_w2(np.ascontiguousarray(W2, np.float32))
    b2 = np.ascontiguousarray(b2, np.float32)
    return [{
        "tv": np.ascontiguousarray(topview[i], np.float32),
        "svb": band_layout(np.ascontiguousarray(search_views[i], np.float32)),
        "W1": W1, "b1": b1, "W2p": W2p, "b2": b2,
    } for i in range(topview.shape[0])]


def kernel(topview, search_views, W1, b1, W2, b2, svh, svw):
    svh, svw = int(svh), int(svw)
    nc = _compiled(B, svh, svw)
    in_maps = make_in_maps(topview, search_views, W1, b1, W2, b2)
    res = bass_utils.run_bass_kernel_spmd(nc, in_maps, core_ids=list(range(B)))
    return np.concatenate([res.results[i]["out"] for i in range(B)], axis=0)

